# revision 1
# baseline (speedup 1.0000x reference)
"""Distributed MLA-style attention on 8 Trainium2 NeuronCores.

Sharding: tensor-parallel over num_heads=24 -> 3 heads per core
(per the sharding hint). Per-core work: shared low-rank projections
(replicated, small), per-head-group attention (scores/softmax/AV), and
the head-sharded slice of o_proj. Partial o_proj outputs are summed
across cores with an on-device all-reduce (psum); the full [B,S,D]
output is returned.
"""
import math

import numpy as np
import jax
import jax.numpy as jnp
from jax.sharding import Mesh, PartitionSpec as P
from jax.experimental.shard_map import shard_map

B, S, D = 4, 2048, 768
H = 24
NOPE, ROPE_D, VD = 32, 16, 32
QHD = NOPE + ROPE_D
QR, KVR = 384, 128
HEAD_DIM = D // H  # 32; softmax scale
NCORES = 8
HLOC = H // NCORES  # 3 heads per core

_SCALE = 1.0 / math.sqrt(HEAD_DIM)


def _rms_norm(x, w, eps=1e-5):
    x32 = x.astype(jnp.float32)
    y = x32 * jax.lax.rsqrt(jnp.mean(x32 * x32, axis=-1, keepdims=True) + eps)
    return y.astype(x.dtype) * w


def _rotate(t, cos, sin):
    # t: [B, S, N, r]; cos/sin: [S, r//2]
    shp = t.shape
    tr = t.astype(jnp.float32).reshape(shp[:-1] + (-1, 2))
    xr, xi = tr[..., 0], tr[..., 1]
    c = cos.reshape(1, shp[1], 1, -1)
    s = sin.reshape(1, shp[1], 1, -1)
    out = jnp.stack([xr * c - xi * s, xr * s + xi * c], axis=-1).reshape(shp)
    return out.astype(t.dtype)


def _body(x, mask, freqs_cos, freqs_sin, Wqa, qa_ln, Wqb_s, Wkva, kv_ln,
          Wkvb_s, Wo_s):
    # Per-shard shapes: Wqb_s [QR, HLOC*QHD], Wkvb_s [KVR, HLOC*(NOPE+VD)],
    # Wo_s [HLOC*VD, D]. Everything else replicated.
    b, s = B, S
    q = _rms_norm(x @ Wqa, qa_ln) @ Wqb_s
    q = q.reshape(b, s, HLOC, QHD).transpose(0, 2, 1, 3)      # [B,hl,S,48]
    q_nope, q_pe = q[..., :NOPE], q[..., NOPE:]
    ckv = x @ Wkva
    c_kv, k_pe = ckv[..., :KVR], ckv[..., KVR:]
    kv = (_rms_norm(c_kv, kv_ln) @ Wkvb_s).reshape(b, s, HLOC, NOPE + VD)
    kv = kv.transpose(0, 2, 1, 3)
    k_nope, v = kv[..., :NOPE], kv[..., NOPE:]                # [B,hl,S,32]
    # Reproduce the reference's swapped-rope exactly: rotated shared k_pe
    # goes on the QUERY side (broadcast over heads); rotated per-head q_pe
    # goes on the KEY side.
    rot_qpe = _rotate(q_pe.transpose(0, 2, 1, 3), freqs_cos, freqs_sin)
    rot_kpe = _rotate(k_pe.reshape(b, s, 1, ROPE_D), freqs_cos, freqs_sin)
    q_pe_f = rot_kpe.transpose(0, 2, 1, 3)                    # [B,1,S,16]
    k_pe_f = rot_qpe.transpose(0, 2, 1, 3)                    # [B,hl,S,16]
    qs = jnp.concatenate(
        [q_nope, jnp.broadcast_to(q_pe_f, (b, HLOC, s, ROPE_D))], axis=-1)
    ks_ = jnp.concatenate([k_nope, k_pe_f], axis=-1)          # [B,hl,S,48]

    outs = []
    for bi in range(b):  # loop batches to bound peak scores memory per core
        scores = jnp.einsum('hqd,hkd->hqk', qs[bi], ks_[bi]) * _SCALE
        scores = scores + mask[0, 0, :s, :s][None]
        attn = jax.nn.softmax(scores.astype(jnp.float32), axis=-1)
        attn = attn.astype(ks_.dtype)
        o = jnp.einsum('hqk,hkd->hqd', attn, v[bi])           # [hl,S,32]
        outs.append(o.transpose(1, 0, 2).reshape(s, HLOC * VD))
    attn_out = jnp.stack(outs, axis=0)                        # [B,S,hl*32]
    partial = attn_out @ Wo_s                                 # [B,S,D]
    return jax.lax.psum(partial, 'h')


_CACHE = {}


def _get_fn():
    if 'fn' in _CACHE:
        return _CACHE['fn']
    devs = jax.devices()[:NCORES]
    mesh = Mesh(np.asarray(devs), ('h',))
    rep = P()
    in_specs = (rep, rep, rep, rep, rep, rep,
                P(None, 'h'),        # Wqb reshaped [QR, H, QHD] -> flat below
                rep, rep,
                P(None, 'h'),        # Wkvb
                P('h', None))        # Wo
    fn = jax.jit(shard_map(_body, mesh=mesh, in_specs=in_specs,
                           out_specs=rep, check_rep=False))
    _CACHE['fn'] = (fn, mesh)
    return _CACHE['fn']


def kernel(x, mask, freqs_cos, freqs_sin, Wqa, qa_ln, Wqb, Wkva, kv_ln,
           Wkvb, Wo):
    fn, mesh = _get_fn()
    # Reorder weight columns so a contiguous split over axis gives whole
    # heads: Wqb [QR, H*QHD] is already head-major; same for Wkvb and Wo.
    out = fn(jnp.asarray(x), jnp.asarray(mask), jnp.asarray(freqs_cos),
             jnp.asarray(freqs_sin), jnp.asarray(Wqa), jnp.asarray(qa_ln),
             jnp.asarray(Wqb), jnp.asarray(Wkva), jnp.asarray(kv_ln),
             jnp.asarray(Wkvb), jnp.asarray(Wo))
    return np.asarray(jax.block_until_ready(out)).astype(np.float32)


if __name__ == '__main__':
    rng = np.random.default_rng(0)
    ins = dict(
        x=rng.standard_normal((B, S, D), np.float32),
        mask=np.zeros((1, 1, S, S), np.float32),
        freqs_cos=rng.random((S, ROPE_D // 2), np.float32),
        freqs_sin=rng.random((S, ROPE_D // 2), np.float32),
        Wqa=rng.standard_normal((D, QR), np.float32) * D ** -0.5,
        qa_ln=np.ones((QR,), np.float32),
        Wqb=rng.standard_normal((QR, H * QHD), np.float32) * QR ** -0.5,
        Wkva=rng.standard_normal((D, KVR + ROPE_D), np.float32) * D ** -0.5,
        kv_ln=np.ones((KVR,), np.float32),
        Wkvb=rng.standard_normal((KVR, H * (NOPE + VD)), np.float32) * KVR ** -0.5,
        Wo=rng.standard_normal((H * VD, D), np.float32) * (H * VD) ** -0.5,
    )
    out = kernel(**ins)
    print('kernel out', out.shape, out.dtype, float(np.abs(out).max()))



# revision 26
# speedup vs baseline: 1.1540x; 1.1540x over previous
"""Distributed MLA-style attention on 8 Trainium2 NeuronCores.

Hand-written Bass/Tile kernel, tensor-parallel over num_heads=24
(3 heads per core). Per core: shared low-rank projections (replicated),
flash-style causal attention for its 3 heads in transposed-score layout
(scoresT[k,q] -> exp -> [V|1] matmul gives outputs + softmax sums with
zero transposes in the inner loop), then an AllToAll that redistributes
attention outputs from head-sharded to row-sharded so each core computes
o_proj for 1/8 of the rows. Host concatenates the 8 row shards.

Matmuls run in bf16 (fp32 accumulation in PSUM); rel tolerance is 2e-2.
Falls back to a jax shard_map implementation if the mask is not the
expected causal mask.
"""
import math
import os
import sys

import numpy as np

for _p in ("/opt/trn_rl_repo",):
    if _p not in sys.path:
        sys.path.insert(0, _p)

import ml_dtypes

B, S, D = 4, 2048, 768
H = 24
NOPE, RD, VD = 32, 16, 32
QHD = NOPE + RD              # 48
QR, KVR = 384, 128
HEAD_DIM = D // H            # 32; softmax scale
NCORES = 8
HL = H // NCORES             # 3 heads per core
R = B * S                    # 8192 rows
RT = R // 128                # 64 row tiles
ST = S // 128                # 16 seq tiles per batch
RSHARD = R // NCORES         # 1024 rows per core for o_proj
SCALE = 1.0 / math.sqrt(HEAD_DIM)
EPS = 1e-5

_CACHE = {}


# ----------------------------------------------------------------------------
# Bass module
# ----------------------------------------------------------------------------

def _build_bass():
    import concourse.bass as bass
    import concourse.mybir as mybir
    import concourse.tile as tile
    from concourse import bacc

    dt = mybir.dt
    f32 = dt.float32
    bf16 = dt.bfloat16
    Alu = mybir.AluOpType
    Act = mybir.ActivationFunctionType

    nc = bacc.Bacc(
        "TRN2",
        target_bir_lowering=False,
        debug=False,
        num_devices=NCORES,
    )

    # Inputs (host pre-tiled so every DMA is a contiguous copy).
    xt_d = nc.dram_tensor("xt", [RT, 128, 6, 128], bf16, kind="ExternalInput")
    wqa_d = nc.dram_tensor("wqa", [128, 6, QR], bf16, kind="ExternalInput")
    wkva_d = nc.dram_tensor("wkva", [128, 6, KVR + RD], bf16, kind="ExternalInput")
    wqb_d = nc.dram_tensor("wqb", [128, 3, HL * QHD], bf16, kind="ExternalInput")
    wkvb_d = nc.dram_tensor("wkvb", [128, HL * (NOPE + VD)], bf16, kind="ExternalInput")
    wo_d = nc.dram_tensor("wo", [128, 6, D], bf16, kind="ExternalInput")
    cos_d = nc.dram_tensor("cosf", [128, ST, 3, 8], bf16, kind="ExternalInput")
    sin_d = nc.dram_tensor("sinf", [128, ST, 3, 8], bf16, kind="ExternalInput")
    ident_d = nc.dram_tensor("ident", [128, 128], bf16, kind="ExternalInput")
    tri_d = nc.dram_tensor("tri", [128, 128], bf16, kind="ExternalInput")
    out_d = nc.dram_tensor("out", [RSHARD, D], f32, kind="ExternalOutput")

    with tile.TileContext(nc) as tc:
        with tc.tile_pool(name="consts", bufs=1) as consts, \
             tc.tile_pool(name="persist", bufs=1) as persist, \
             tc.tile_pool(name="dram", bufs=1, space="DRAM") as dram:

            # --- constants -> SBUF
            wqa_sb = consts.tile([128, 6, QR], bf16)
            nc.sync.dma_start(wqa_sb, wqa_d.ap())
            wkva_sb = consts.tile([128, 6, KVR + RD], bf16)
            nc.sync.dma_start(wkva_sb, wkva_d.ap())
            wqb_sb = consts.tile([128, 3, HL * QHD], bf16)
            nc.sync.dma_start(wqb_sb, wqb_d.ap())
            wkvb_sb = consts.tile([128, HL * (NOPE + VD)], bf16)
            nc.sync.dma_start(wkvb_sb, wkvb_d.ap())
            wo_sb = consts.tile([128, 6, D], bf16)
            nc.sync.dma_start(wo_sb, wo_d.ap())
            cos_sb = consts.tile([128, ST, 3, 8], bf16)
            nc.sync.dma_start(cos_sb, cos_d.ap())
            sin_sb = consts.tile([128, ST, 3, 8], bf16)
            nc.sync.dma_start(sin_sb, sin_d.ap())

            ident = consts.tile([128, 128], bf16)
            nc.sync.dma_start(ident, ident_d.ap())
            tri = consts.tile([128, 128], bf16)
            nc.sync.dma_start(tri, tri_d.ap())
            eps_sb = consts.tile([128, 1], f32)
            nc.vector.memset(eps_sb, EPS)

            # --- persistent activation buffers
            # qsT/ksT: per-head transposed [48, cols] vectors, packed two
            # heads per 96 partitions x 2 free slots (head 2 in slot 1).
            # matmul operands must start at partition 0/32/64 -> pack the
            # two heads per slot at partitions 0 and 64
            qsT = persist.tile([112, 2, R], bf16)
            ksT = persist.tile([112, 2, R], bf16)
            # v1: [V_h | 1] per (k-tile, head): AV stationary operand.
            v1 = persist.tile([128, RT, HL, VD + 1], bf16)
            nc.vector.memset(v1[:, :, :, VD:VD + 1], 1.0)

            def qsT_h(h):
                return qsT[(h % 2) * 64:(h % 2) * 64 + 48, h // 2, :]

            def ksT_h(h):
                return ksT[(h % 2) * 64:(h % 2) * 64 + 48, h // 2, :]

            # ------------------------------------------------------------
            # Phase 1: projections + rope + transposes, per 128-row tile
            # ------------------------------------------------------------
            with tc.tile_pool(name="p1sbuf", bufs=3) as p1, \
                 tc.tile_pool(name="p1small", bufs=4) as p1s, \
                 tc.tile_pool(name="mmproj", bufs=4, space="PSUM") as mmpool, \
                 tc.tile_pool(name="tppsum", bufs=4, space="PSUM") as tppool:

                for rt in range(RT):
                    st = rt % ST
                    xt = p1.tile([128, 6, 128], bf16, tag="xt")
                    nc.sync.dma_start(xt, xt_d.ap()[rt])

                    ps_xa = mmpool.tile([128, QR], f32, tag="mm")
                    for c in range(6):
                        nc.tensor.matmul(ps_xa, xt[:, c, :], wqa_sb[:, c, :],
                                         start=(c == 0), stop=(c == 5))
                    ps_ckv = mmpool.tile([128, KVR + RD], f32, tag="mm")
                    for c in range(6):
                        nc.tensor.matmul(ps_ckv, xt[:, c, :], wkva_sb[:, c, :],
                                         start=(c == 0), stop=(c == 5))

                    # single copy of each psum tensor to SBUF bf16; all
                    # downstream DVE ops read SBUF (psum-dual-read is illegal)
                    xa_sb = p1.tile([128, QR], bf16, tag="xa_sb")
                    nc.vector.tensor_copy(xa_sb, ps_xa)
                    ck_sb = p1.tile([128, KVR + RD], bf16, tag="ck_sb")
                    nc.vector.tensor_copy(ck_sb, ps_ckv)

                    # rms statistics (mean of squares) for xa and c_kv
                    sq_a = p1.tile([128, QR], f32, tag="sq")
                    msq = p1s.tile([128, 2], f32, tag="msq")
                    nc.vector.tensor_mul(sq_a, xa_sb, xa_sb)
                    nc.vector.tensor_reduce(out=msq[:, 0:1], in_=sq_a,
                                            axis=mybir.AxisListType.X,
                                            op=Alu.add)
                    sq_b = p1.tile([128, KVR], f32, tag="sq2")
                    nc.vector.tensor_mul(sq_b, ck_sb[:, :KVR], ck_sb[:, :KVR])
                    nc.vector.tensor_reduce(out=msq[:, 1:2], in_=sq_b,
                                            axis=mybir.AxisListType.X,
                                            op=Alu.add)
                    rstd = p1s.tile([128, 2], f32, tag="rstd")
                    # sqrt(sum/N + eps): fold 1/N into the activation scale
                    nc.scalar.activation(rstd[:, 0:1], msq[:, 0:1], Act.Sqrt,
                                         bias=eps_sb, scale=1.0 / QR)
                    nc.scalar.activation(rstd[:, 1:2], msq[:, 1:2], Act.Sqrt,
                                         bias=eps_sb, scale=1.0 / KVR)
                    nc.vector.reciprocal(rstd, rstd)

                    qlat = p1.tile([128, QR], bf16, tag="qlat")
                    nc.vector.tensor_scalar_mul(qlat, xa_sb, rstd[:, 0:1])
                    ckvn = p1.tile([128, KVR], bf16, tag="ckvn")
                    nc.vector.tensor_scalar_mul(ckvn, ck_sb[:, :KVR], rstd[:, 1:2])

                    # rope(k_pe): shared across heads, goes to the QUERY side
                    kr = ck_sb[:, KVR:KVR + 8]
                    ki = ck_sb[:, KVR + 8:KVR + 16]
                    cosr = cos_sb[:, st, 0, :]
                    sinr = sin_sb[:, st, 0, :]
                    t1 = p1s.tile([128, 8], f32, tag="t1")
                    t2 = p1s.tile([128, 8], f32, tag="t2")
                    rkr = p1s.tile([128, 8], bf16, tag="rkr")
                    rki = p1s.tile([128, 8], bf16, tag="rki")
                    nc.vector.tensor_mul(t1, kr, cosr)
                    nc.vector.tensor_mul(t2, ki, sinr)
                    nc.vector.tensor_sub(rkr, t1, t2)
                    nc.vector.tensor_mul(t1, kr, sinr)
                    nc.vector.tensor_mul(t2, ki, cosr)
                    nc.vector.tensor_add(rki, t1, t2)

                    # transpose qlat (3 blocks) and ckvn for the next matmuls
                    tp_q = tppool.tile([128, 3, 128], bf16, tag="tp")
                    for c in range(3):
                        nc.tensor.transpose(tp_q[:, c, :],
                                            qlat[:, c * 128:(c + 1) * 128], ident)
                    qlatT = p1.tile([128, 3, 128], bf16, tag="qlatT")
                    nc.vector.tensor_copy(qlatT, tp_q)
                    tp_k = tppool.tile([128, 128], bf16, tag="tp")
                    nc.tensor.transpose(tp_k, ckvn, ident)
                    ckvT = p1.tile([128, 128], bf16, tag="ckvT")
                    nc.vector.tensor_copy(ckvT, tp_k)

                    ps_q = mmpool.tile([128, HL * QHD], f32, tag="mm")
                    for c in range(3):
                        nc.tensor.matmul(ps_q, qlatT[:, c, :], wqb_sb[:, c, :],
                                         start=(c == 0), stop=(c == 2))
                    ps_kv = mmpool.tile([128, HL * (NOPE + VD)], f32, tag="mm")
                    nc.tensor.matmul(ps_kv, ckvT, wkvb_sb, start=True, stop=True)

                    q_sb = p1.tile([128, HL, QHD], bf16, tag="q_sb")
                    nc.vector.tensor_copy(
                        q_sb, ps_q.rearrange("p (h d) -> p h d", h=HL))
                    kv_sb = p1.tile([128, HL, NOPE + VD], bf16, tag="kv_sb")
                    nc.vector.tensor_copy(
                        kv_sb, ps_kv.rearrange("p (h d) -> p h d", h=HL))

                    # query-side staging: [q_nope_h | rot_kpe] per head
                    qs_st = p1.tile([128, HL, QHD], bf16, tag="qs_st")
                    nc.vector.tensor_copy(qs_st[:, :, 0:NOPE], q_sb[:, :, 0:NOPE])
                    for h in range(HL):
                        nc.vector.tensor_copy(qs_st[:, h, NOPE:NOPE + 8], rkr)
                        nc.vector.tensor_copy(qs_st[:, h, NOPE + 8:QHD], rki)

                    # key-side staging: [k_nope_h | rope(q_pe_h)]
                    ks_st = p1.tile([128, HL, QHD], bf16, tag="ks_st")
                    nc.vector.tensor_copy(ks_st[:, :, 0:NOPE], kv_sb[:, :, 0:NOPE])
                    qpr = q_sb[:, :, NOPE:NOPE + 8]
                    qpi = q_sb[:, :, NOPE + 8:QHD]
                    cos3 = cos_sb[:, st, :, :]
                    sin3 = sin_sb[:, st, :, :]
                    t3 = p1s.tile([128, 3, 8], f32, tag="t3")
                    t4 = p1s.tile([128, 3, 8], f32, tag="t4")
                    nc.vector.tensor_mul(t3, qpr, cos3)
                    nc.vector.tensor_mul(t4, qpi, sin3)
                    nc.vector.tensor_sub(ks_st[:, :, NOPE:NOPE + 8], t3, t4)
                    nc.vector.tensor_mul(t3, qpr, sin3)
                    nc.vector.tensor_mul(t4, qpi, cos3)
                    nc.vector.tensor_add(ks_st[:, :, NOPE + 8:QHD], t3, t4)

                    # v1 slices
                    nc.vector.tensor_copy(v1[:, rt, :, 0:VD],
                                          kv_sb[:, :, NOPE:NOPE + VD])

                    # transpose staged q/k vectors into qsT/ksT
                    tp_qs = tppool.tile([48, HL, 128], bf16, tag="tp")
                    for h in range(HL):
                        nc.tensor.transpose(tp_qs[:, h, :], qs_st[:, h, :], ident)
                    tp_ks = tppool.tile([48, HL, 128], bf16, tag="tp")
                    for h in range(HL):
                        nc.tensor.transpose(tp_ks[:, h, :], ks_st[:, h, :], ident)
                    for h in range(HL):
                        nc.vector.tensor_copy(
                            qsT_h(h)[:, rt * 128:(rt + 1) * 128], tp_qs[:, h, :])
                        nc.vector.tensor_copy(
                            ksT_h(h)[:, rt * 128:(rt + 1) * 128], tp_ks[:, h, :])

            # ------------------------------------------------------------
            # Phase 2: causal attention per (batch, head), scoresT layout
            # ------------------------------------------------------------
            a2a_in = dram.tile([NCORES * HL * VD, RSHARD], bf16)
            a2a_out = dram.tile([NCORES * HL * VD, RSHARD], bf16)

            with tc.tile_pool(name="att", bufs=3) as att, \
                 tc.tile_pool(name="atts", bufs=3) as atts, \
                 tc.tile_pool(name="ps_s", bufs=3, space="PSUM") as pss_pool, \
                 tc.tile_pool(name="ps_o", bufs=2, space="PSUM") as pso_pool:

                for b in range(B):
                    base = b * S
                    for h in range(HL):
                        qT = qsT_h(h)
                        kT = ksT_h(h)
                        for qg in range(4):
                            ps_o = pso_pool.tile([VD + 1, 512], f32, tag="pso")
                            nkt = 4 * (qg + 1)
                            for kj in range(nkt):
                                ps_s = pss_pool.tile([128, 512], f32, tag="pss")
                                nc.tensor.matmul(
                                    ps_s,
                                    kT[:, base + kj * 128:base + (kj + 1) * 128],
                                    qT[:, base + qg * 512:base + (qg + 1) * 512],
                                    start=True, stop=True)
                                pT = att.tile([128, 512], bf16, tag="pT")
                                nc.scalar.activation(pT, ps_s, Act.Exp,
                                                     scale=SCALE)
                                if kj >= 4 * qg:
                                    dd = kj * 128 - qg * 512
                                    if dd > 0:
                                        nc.vector.memset(pT[:, :dd], 0.0)
                                    nc.vector.tensor_mul(
                                        pT[:, dd:dd + 128], pT[:, dd:dd + 128],
                                        tri)
                                nc.tensor.matmul(
                                    ps_o, v1[:, b * ST + kj, h, :], pT,
                                    start=(kj == 0), stop=(kj == nkt - 1))
                            # normalize: rows 0:VD are sum(p*v), row VD is sum(p)
                            srow = atts.tile([1, 512], f32, tag="srow")
                            nc.vector.reciprocal(srow, ps_o[VD:VD + 1, :])
                            sd = dram.tile([1, 512], f32, tag="sd", bufs=3)
                            nc.sync.dma_start(sd, srow)
                            sbc = atts.tile([VD, 512], f32, tag="sbc")
                            bsrc = bass.AP(tensor=sd.tensor, offset=sd.offset,
                                           ap=[[0, VD], [1, 512]])
                            nc.sync.dma_start(sbc, bsrc)
                            oTn = att.tile([VD, 512], bf16, tag="oTn")
                            nc.vector.tensor_mul(oTn, ps_o[0:VD, :], sbc)
                            g0 = base + qg * 512
                            shard, off = divmod(g0, RSHARD)
                            nc.sync.dma_start(
                                a2a_in[shard * HL * VD + h * VD:
                                       shard * HL * VD + (h + 1) * VD,
                                       off:off + 512],
                                oTn)

            # ------------------------------------------------------------
            # Phase 3: AllToAll (head-sharded -> row-sharded) + o_proj
            # ------------------------------------------------------------
            nc.gpsimd.collective_compute(
                "AllToAll",
                mybir.AluOpType.bypass,
                replica_groups=[list(range(NCORES))],
                ins=[a2a_in.opt()],
                outs=[a2a_out.opt()],
            )

            with tc.tile_pool(name="op", bufs=2) as op, \
                 tc.tile_pool(name="ps_op", bufs=2, space="PSUM") as psop_pool:
                oT_all = op.tile([128, 6, RSHARD], bf16, tag="oT_all")
                nc.sync.dma_start(
                    oT_all, a2a_out.rearrange("(c p) n -> p c n", p=128))
                for m in range(RSHARD // 128):
                    ps = psop_pool.tile([128, D], f32, tag="psop")
                    for c in range(6):
                        lhsT = oT_all[:, c, m * 128:(m + 1) * 128]
                        nc.tensor.matmul(ps[:, 0:512], lhsT, wo_sb[:, c, 0:512],
                                         start=(c == 0), stop=(c == 5))
                    for c in range(6):
                        lhsT = oT_all[:, c, m * 128:(m + 1) * 128]
                        nc.tensor.matmul(ps[:, 512:D], lhsT, wo_sb[:, c, 512:D],
                                         start=(c == 0), stop=(c == 5))
                    ot = op.tile([128, D], f32, tag="ot")
                    nc.vector.tensor_copy(ot, ps)
                    nc.sync.dma_start(out_d.ap()[m * 128:(m + 1) * 128, :], ot)

    nc.compile()
    return nc


# ----------------------------------------------------------------------------
# Host-side input prep
# ----------------------------------------------------------------------------

def _bf16(a):
    return np.ascontiguousarray(a.astype(ml_dtypes.bfloat16))


_PE_PERM = np.concatenate([
    np.arange(NOPE),
    NOPE + 2 * np.arange(8),       # real parts
    NOPE + 2 * np.arange(8) + 1,   # imag parts
])


def _prep_common(x, freqs_cos, freqs_sin, Wqa, Wkva):
    # x -> pre-tiled transposed tiles [RT, 128(d-in-chunk), 6(chunk), 128(row)]
    x2 = np.asarray(x, np.float32).reshape(RT, 128, 6, 128)  # [rt, j, c, p]
    xt = _bf16(np.transpose(x2, (0, 3, 2, 1)))
    wqa = _bf16(np.asarray(Wqa, np.float32).reshape(6, 128, QR)
                .transpose(1, 0, 2))
    wkva_p = np.asarray(Wkva, np.float32).copy()
    pe = wkva_p[:, KVR:].copy()
    wkva_p[:, KVR + 0:KVR + 8] = pe[:, 0::2]
    wkva_p[:, KVR + 8:KVR + 16] = pe[:, 1::2]
    wkva = _bf16(wkva_p.reshape(6, 128, KVR + RD).transpose(1, 0, 2))
    cos3 = np.repeat(np.asarray(freqs_cos, np.float32)[:, None, :], 3, axis=1)
    sin3 = np.repeat(np.asarray(freqs_sin, np.float32)[:, None, :], 3, axis=1)
    cosf = _bf16(cos3.reshape(ST, 128, 3, 8).transpose(1, 0, 2, 3))
    sinf = _bf16(sin3.reshape(ST, 128, 3, 8).transpose(1, 0, 2, 3))
    return xt, wqa, wkva, cosf, sinf


def _prep_consts():
    ident = _bf16(np.eye(128, dtype=np.float32))
    k = np.arange(128)[:, None]
    q = np.arange(128)[None, :]
    tri = _bf16((q >= k).astype(np.float32))
    return ident, tri


def _prep_core(c, Wqb_ln, Wkvb_ln):
    # per-head column slice + pe de-interleave for Wqb
    wqb_h = Wqb_ln.reshape(QR, H, QHD)[:, c * HL:(c + 1) * HL, :]
    wqb_h = wqb_h[:, :, _PE_PERM].reshape(QR, HL * QHD)
    wqb = _bf16(wqb_h.reshape(3, 128, HL * QHD).transpose(1, 0, 2))
    wkvb = _bf16(Wkvb_ln.reshape(KVR, H, NOPE + VD)
                 [:, c * HL:(c + 1) * HL, :].reshape(KVR, HL * (NOPE + VD)))
    return wqb, wkvb


def _mask_is_causal(mask):
    m = np.asarray(mask).reshape(S, S)
    tril = np.tril(np.ones((S, S), bool))
    if not np.all(m[tril] == 0.0):
        return False
    upper = m[~tril]
    return np.all(upper <= -1e8)


# ----------------------------------------------------------------------------
# Compiled runner (mirrors bass2jax.run_bass_via_pjrt, cached, no donation)
# ----------------------------------------------------------------------------

def _get_runner():
    if "runner" in _CACHE:
        return _CACHE["runner"]
    import jax
    from jax.sharding import Mesh, PartitionSpec as P
    from jax.experimental.shard_map import shard_map
    import concourse.mybir as mybir
    from concourse import bass2jax

    nc = _build_bass()
    bass2jax.install_neuronx_cc_hook()

    partition_name = (nc.partition_id_tensor.name
                      if nc.partition_id_tensor else None)
    in_names, out_names, out_avals, zero_shapes = [], [], [], []
    for alloc in nc.m.functions[0].allocations:
        if not isinstance(alloc, mybir.MemoryLocationSet):
            continue
        name = alloc.memorylocations[0].name
        if alloc.kind == "ExternalInput":
            if name != partition_name:
                in_names.append(name)
        elif alloc.kind == "ExternalOutput":
            out_names.append(name)
            shape = tuple(alloc.tensor_shape)
            dtype = mybir.dt.np(alloc.dtype)
            out_avals.append(jax.core.ShapedArray(shape, dtype))
            zero_shapes.append((shape, dtype))
    n_params = len(in_names)
    all_in_names = in_names + out_names
    if partition_name is not None:
        all_in_names = all_in_names + [partition_name]

    def _body(*args):
        operands = list(args)
        if partition_name is not None:
            operands.append(bass2jax.partition_id_tensor())
        outs = bass2jax._bass_exec_p.bind(
            *operands,
            out_avals=tuple(out_avals),
            in_names=tuple(all_in_names),
            out_names=tuple(out_names),
            lowering_input_output_aliases=(),
            sim_require_finite=True,
            sim_require_nnan=True,
            nc=nc,
        )
        return tuple(outs)

    devices = jax.devices()[:NCORES]
    mesh = Mesh(np.asarray(devices), ("core",))
    n_all = n_params + len(out_names)
    sharded = jax.jit(
        shard_map(_body, mesh=mesh, in_specs=(P("core"),) * n_all,
                  out_specs=(P("core"),) * len(out_names), check_rep=False),
        keep_unused=True,
    )
    _CACHE["runner"] = (sharded, in_names, out_names, zero_shapes)
    return _CACHE["runner"]


def _run_bass(per_core_inputs):
    import jax
    sharded, in_names, out_names, zero_shapes = _get_runner()
    concat_in = [
        np.concatenate([per_core_inputs[c][name] for c in range(NCORES)], axis=0)
        for name in in_names
    ]
    zeros = [np.zeros((NCORES * sh[0], *sh[1:]), dt) for sh, dt in zero_shapes]
    outs = sharded(*concat_in, *zeros)
    out = np.asarray(jax.block_until_ready(outs[0]))
    return out  # [NCORES*RSHARD, D] in row order


# ----------------------------------------------------------------------------
# Public entry
# ----------------------------------------------------------------------------

def prep_all(x, freqs_cos, freqs_sin, Wqa, qa_ln, Wqb, Wkva, kv_ln, Wkvb, Wo):
    """Build the 8 per-core input dicts for the bass kernel."""
    xt, wqa, wkva, cosf, sinf = _prep_common(x, freqs_cos, freqs_sin, Wqa, Wkva)
    Wqb_ln = np.asarray(Wqb, np.float32) * np.asarray(qa_ln, np.float32)[:, None]
    Wkvb_ln = np.asarray(Wkvb, np.float32) * np.asarray(kv_ln, np.float32)[:, None]
    wo = _bf16(np.asarray(Wo, np.float32).reshape(6, 128, D).transpose(1, 0, 2))
    ident, tri = _prep_consts()
    per_core = []
    for c in range(NCORES):
        wqb_c, wkvb_c = _prep_core(c, Wqb_ln, Wkvb_ln)
        per_core.append(dict(xt=xt, wqa=wqa, wkva=wkva, wqb=wqb_c,
                             wkvb=wkvb_c, wo=wo, cosf=cosf, sinf=sinf,
                             ident=ident, tri=tri))
    return per_core


def kernel(x, mask, freqs_cos, freqs_sin, Wqa, qa_ln, Wqb, Wkva, kv_ln,
           Wkvb, Wo):
    if not _mask_is_causal(mask):
        return _fallback_jax(x, mask, freqs_cos, freqs_sin, Wqa, qa_ln, Wqb,
                             Wkva, kv_ln, Wkvb, Wo)

    per_core = prep_all(x, freqs_cos, freqs_sin, Wqa, qa_ln, Wqb, Wkva,
                        kv_ln, Wkvb, Wo)
    out = _run_bass(per_core)
    return np.ascontiguousarray(out.reshape(B, S, D)).astype(np.float32)


# ----------------------------------------------------------------------------
# Fallback: jax shard_map implementation (only for unexpected masks)
# ----------------------------------------------------------------------------

def _fallback_jax(x, mask, freqs_cos, freqs_sin, Wqa, qa_ln, Wqb, Wkva, kv_ln,
                  Wkvb, Wo):
    import jax
    import jax.numpy as jnp
    from jax.sharding import Mesh, PartitionSpec as P
    from jax.experimental.shard_map import shard_map

    def _rms_norm(x_, w, eps=1e-5):
        x32 = x_.astype(jnp.float32)
        y = x32 * jax.lax.rsqrt(jnp.mean(x32 * x32, axis=-1, keepdims=True) + eps)
        return y.astype(x_.dtype) * w

    def _rotate(t, cos, sin):
        shp = t.shape
        tr = t.astype(jnp.float32).reshape(shp[:-1] + (-1, 2))
        xr, xi = tr[..., 0], tr[..., 1]
        c = cos.reshape(1, shp[1], 1, -1)
        s = sin.reshape(1, shp[1], 1, -1)
        out = jnp.stack([xr * c - xi * s, xr * s + xi * c], axis=-1).reshape(shp)
        return out.astype(t.dtype)

    def _body(x_, mask_, fc, fs, wqa_, qaln_, wqb_s, wkva_, kvln_, wkvb_s, wo_s):
        b, s = B, S
        q = _rms_norm(x_ @ wqa_, qaln_) @ wqb_s
        q = q.reshape(b, s, HL, QHD).transpose(0, 2, 1, 3)
        q_nope, q_pe = q[..., :NOPE], q[..., NOPE:]
        ckv = x_ @ wkva_
        c_kv, k_pe = ckv[..., :KVR], ckv[..., KVR:]
        kv = (_rms_norm(c_kv, kvln_) @ wkvb_s).reshape(b, s, HL, NOPE + VD)
        kv = kv.transpose(0, 2, 1, 3)
        k_nope, v = kv[..., :NOPE], kv[..., NOPE:]
        rot_qpe = _rotate(q_pe.transpose(0, 2, 1, 3), fc, fs)
        rot_kpe = _rotate(k_pe.reshape(b, s, 1, RD), fc, fs)
        q_pe_f = rot_kpe.transpose(0, 2, 1, 3)
        k_pe_f = rot_qpe.transpose(0, 2, 1, 3)
        qs = jnp.concatenate(
            [q_nope, jnp.broadcast_to(q_pe_f, (b, HL, s, RD))], axis=-1)
        ks_ = jnp.concatenate([k_nope, k_pe_f], axis=-1)
        outs = []
        for bi in range(b):
            scores = jnp.einsum('hqd,hkd->hqk', qs[bi], ks_[bi]) * SCALE
            scores = scores + mask_[0, 0, :s, :s][None]
            attn = jax.nn.softmax(scores.astype(jnp.float32), axis=-1)
            attn = attn.astype(ks_.dtype)
            o = jnp.einsum('hqk,hkd->hqd', attn, v[bi])
            outs.append(o.transpose(1, 0, 2).reshape(s, HL * VD))
        attn_out = jnp.stack(outs, axis=0)
        partial = attn_out @ wo_s
        return jax.lax.psum(partial, 'h')

    if "fallback" not in _CACHE:
        devs = jax.devices()[:NCORES]
        mesh = Mesh(np.asarray(devs), ('h',))
        rep = P()
        in_specs = (rep, rep, rep, rep, rep, rep,
                    P(None, 'h'), rep, rep, P(None, 'h'), P('h', None))
        fn = jax.jit(shard_map(_body, mesh=mesh, in_specs=in_specs,
                               out_specs=rep, check_rep=False))
        _CACHE["fallback"] = fn
    fn = _CACHE["fallback"]
    import jax.numpy as jnp
    out = fn(jnp.asarray(x), jnp.asarray(mask), jnp.asarray(freqs_cos),
             jnp.asarray(freqs_sin), jnp.asarray(Wqa), jnp.asarray(qa_ln),
             jnp.asarray(Wqb), jnp.asarray(Wkva), jnp.asarray(kv_ln),
             jnp.asarray(Wkvb), jnp.asarray(Wo))
    return np.asarray(jax.block_until_ready(out)).astype(np.float32)


if __name__ == '__main__':
    rng = np.random.default_rng(0)
    ins = dict(
        x=rng.standard_normal((B, S, D), np.float32),
        mask=np.where(np.tril(np.ones((S, S), bool)), 0.0, -1e9)
            .astype(np.float32)[None, None],
        freqs_cos=rng.random((S, RD // 2), np.float32),
        freqs_sin=rng.random((S, RD // 2), np.float32),
        Wqa=rng.standard_normal((D, QR), np.float32) * D ** -0.5,
        qa_ln=np.ones((QR,), np.float32),
        Wqb=rng.standard_normal((QR, H * QHD), np.float32) * QR ** -0.5,
        Wkva=rng.standard_normal((D, KVR + RD), np.float32) * D ** -0.5,
        kv_ln=np.ones((KVR,), np.float32),
        Wkvb=rng.standard_normal((KVR, H * (NOPE + VD)), np.float32) * KVR ** -0.5,
        Wo=rng.standard_normal((H * VD, D), np.float32) * (H * VD) ** -0.5,
    )
    out = kernel(**ins)
    print('kernel out', out.shape, out.dtype, float(np.abs(out).max()))
